# revision 70
# baseline (speedup 1.0000x reference)
"""Trainium2 Bass kernel for the SG-visibility sampling network (v2).

Math notes (exploited structure):
  - U,V are orthogonal to the unit lobe axis l, so dot(sample_dir, l) == cos(r_phi)
    exactly.  Hence the SG weight w = exp(sharp*(cos_phi-1)) is a per-lobe
    constant and sum_s(vis*w)/(sum_s w + TINY) = scale_l * sum_s vis with
    scale_l = w/(S*w + TINY), precomputed on host.
  - pre-activation of the hidden layer decomposes as
        pre_h[n,l,s,h] = P_n[h] - C_l[h] - ct[n,l,s]*A_l[h] - st[n,l,s]*B_l[h]
    with P_n = p_n @ W1[:3] + b1,  A_l = sp_l*(U_l@Wd),  B_l = sp_l*(V_l@Wd),
    C_l = cp_l*(l_l@Wd),  Wd = root_rot @ W1[3:].
  - hemisphere mask: cos_term = ct*a_nl + st*b_nl + c_nl with
    a = normals@(sp*U)_l, b = normals@(sp*V)_l, c = normals@(cp*l)_l.
  - sigmoid(z) = 0.5*tanh(z/2) + 0.5, so vis*msk = 0.5*(tanh+1)*msk and the
    weighted sum runs as ONE accumulating matmul per (chunk, half) with
    scale_l/2 stationary.  Using Tanh instead of Sigmoid keeps every
    activation (Sin/Tanh/Relu/Copy) in ONE ACT table set (silu_and_others)
    -- zero LoadActFuncSet switches in steady state (the table-map cache is
    narrowed in _build_program so the greedy placement pass must pick it).

Device schedule (per core, data-parallel over N; HW-profiled at ~196us/core
vs the 275us baseline):
  - mask path in full fp32 (sign-exact): the a/b/c dots are HOST-precomputed
    in f64, s-duplicated, and DMA'd per chunk on the scalar HWDGE queue
    (c' = TINY - c so the compare is one is_gt); q1/q2/q3/cmp + (tanh+1)*msk
    run on DVE, drip-fed one-per-zmm into the NEXT half's stream.
  - hidden path in bf16 with PE ROW TILING: each chunk ships as ONE
    [128, 6144] "chunkbuf" DMA (12KB descriptor rows, ~4x queue bandwidth
    vs 2KB rows) holding four 4-lobe group tiles + the wcst stationary
    bands.  A lobe's K-band is 32 rows [theta-cos8 | theta-sin8 |
    asin-coded pc4 | zeros] at array band 32*j', so the four lobes of a
    group run CONCURRENTLY in the 128x128 array (tile_position row tiling,
    ~3x hidden speedup).  ONE in-place Sin per group tile recovers ct|st
    and decodes the asin-coded point rows (Sin(asin(p/PS)) = p/PS; wcst
    carries the PS scale).
  - z: per-lobe K=128 bf16 matmul against block-diag W2 (relu drains
    alternate ACT/DVE, order flipped on half 1 so drains never queue
    behind the block Sins), one tanh per half, and the scale matmul into
    the [L, n] output PSUM deferred into the next half's stream.
  - Deferred-emission schedule: each half's mask chain, mask trig, next
    chunk's block Sins, and sum matmul are emitted one-per-zmm-iteration
    into the NEXT half, so ACT/DVE FIFOs never burst at a half boundary.

Measured dead ends (do not revisit without new evidence): fp8 z-matmul
(e4m3 hr quantization alone costs 3.9e-2 rel err vs the 2e-2 gate; hi+lo
residual split fixes precision but any extra full pass over the 16.8M
hidden activations costs ~137us on a helper engine); zmm col-pair tiling
(pairs do overlap in the array but drain supply becomes the bottleneck);
[128,1024] pair drains (drain latency then gates zmm starts).
"""

import numpy as np

N, L, S, H = 8192, 128, 8, 16
NCORES = 8
NC = N // NCORES          # rays per core
LPC = 16                  # lobes per chunk
CHUNKS = L // LPC
TINY = 1e-6
HF = NC // 2              # moving-operand free-dim limit (512)

# inp (f32) row layout
R_RT = 0                  # rows 0..1023: r_theta in [l*S+s, n] layout
R_CB = L * S              # rows 1024..1151: cb [128, 8] in cols 0..7
INP_ROWS = R_CB + 128

# big (bf16, 1D): per chunk ONE [128, CB_COLS] "chunkbuf" holding four
# 4-lobe group tiles AND the wcst stationary bands side by side, so each
# partition row is a 12KB contiguous run -- DMA descriptor efficiency is
# ~4x better than per-tile 2KB rows (measured 28 -> ~113 GB/s per queue).
# cols: [grp0 | grp1 | grp2 | grp3 | wcst] = [1024 x4 | 2048]
#
# Each group tile packs 4 lobes into 32-row K-bands (rows 32*j' + k):
#   k 0..7 = theta rows (cos phase), 8..15 = theta rows again (sin phase),
#   16..18 = asin(points/PS)*4/pi, 19 = asin(1/PS)*4/pi, 20..31 = zeros.
# ONE in-place Sin per group tile recovers ct|st AND decodes the asin-coded
# point rows (Sin(asin(p/PS)) = p/PS; the wcst rows carry the PS scale), so
# the per-lobe hidden matmul is a 32x128 row-tile at array band 32*j' --
# four lobes' matmuls run CONCURRENTLY in the PE array (row tiling).
CB_COLS = 4 * NC + LPC * 128       # 6144
O_BLK = 0
O_WSIG = O_BLK + CHUNKS * 128 * CB_COLS
O_WSUM = O_WSIG + 128 * 512
BIG_ELEMS = O_WSUM + 128 * CHUNKS * 128

_PROG = None


def _build_program():
    import concourse.bass as bass
    import concourse.bacc as bacc
    import concourse.mybir as mybir
    import concourse.tile as tile

    f32 = mybir.dt.float32
    bf16 = mybir.dt.bfloat16
    AF = mybir.ActivationFunctionType
    ALU = mybir.AluOpType
    PI4 = float(np.pi / 4.0)

    nc = bacc.Bacc("TRN2", target_bir_lowering=False, debug=False,
                   num_devices=NCORES)

    # The act-table-load pass greedily picks the FIRST table set containing
    # each activation's func, which alternates trig_and_small <-> exp_and_others
    # for Sin/Tanh (38 reloads, ~50us serialized on ACT).  silu_and_others
    # genuinely contains Sin+Tanh+Relu together; constrain the (cached) table
    # map so the pass can only pick it for Sin/Tanh.  Set ids are positional,
    # so entries are mutated in place -- never reordered.
    from concourse.hw_specs import get_activation_tables
    tabs = get_activation_tables(nc.m.arch)
    assert {AF.Sin, AF.Tanh, AF.Relu} <= tabs["silu_and_others"]
    for name, funcs in tabs.items():
        if name != "silu_and_others":
            funcs.discard(AF.Sin)
            funcs.discard(AF.Tanh)

    inp = nc.declare_dram_parameter("inp", [INP_ROWS, NC], f32, isOutput=False)
    big = nc.declare_dram_parameter("big", [BIG_ELEMS], bf16, isOutput=False)
    # host-prebaked s-duplicated hemisphere-mask dots, [128,(a|b|c')] per chunk
    abc = nc.declare_dram_parameter("abc", [CHUNKS * 128, 3 * NC], f32,
                                    isOutput=False)
    out = nc.declare_dram_parameter("out", [L, NC], f32, isOutput=True)

    def bslice(off, p, c):
        return big[off:off + p * c].rearrange("(p c) -> p c", p=p, c=c)

    # relu-drain engine per lobe-in-chunk (Pool/gpsimd cannot read PSUM, so
    # drains alternate ACT / DVE; Pool owns the SBUF-only mask chain)
    DRAIN = "ADADADADADADADAD"
    assert len(DRAIN) == LPC

    with tile.TileContext(nc) as tc:
        with (
            tc.tile_pool(name="const", bufs=1) as cpool,
            tc.tile_pool(name="io", bufs=3) as io,
            tc.tile_pool(name="trig", bufs=2) as trig,
            tc.tile_pool(name="abc", bufs=3) as abcp,
            tc.tile_pool(name="work", bufs=3) as work,
            tc.tile_pool(name="hrp", bufs=9) as hrp,
            tc.tile_pool(name="ps", bufs=4, space=bass.MemorySpace.PSUM) as ps,
            tc.tile_pool(name="zps", bufs=2, space=bass.MemorySpace.PSUM) as zps,
            tc.tile_pool(name="ops", bufs=1, space=bass.MemorySpace.PSUM) as opsp,
        ):
            # trigger the silu_and_others ACT table load during the startup
            # DMA window instead of right before the first real Sin
            warm = cpool.tile([1, 1], f32)
            nc.gpsimd.memset(warm[:], 0.0)
            nc.scalar.activation(warm[:], warm[:], AF.Sin)

            cb_t = cpool.tile([128, 8], f32)
            nc.sync.dma_start(cb_t[:], inp[R_CB:R_CB + 128, 0:8])

            out_ps = opsp.tile([128, NC], f32)

            def issue_loads(C):
                t = {}
                # one chunkbuf DMA per chunk (10KB descriptors); chunk 0
                # splits it so the blk0 Sin + first hidden matmuls can start
                # before the wcst tail lands.
                cbuf = io.tile([128, CB_COLS], bf16, tag="cbuf")
                src = big[O_BLK + C * 128 * CB_COLS:
                          O_BLK + (C + 1) * 128 * CB_COLS]
                src = src.rearrange("(p c) -> p c", p=128, c=CB_COLS)
                if C == 0:
                    # group 0 + its wcst bands first: gates the very first
                    # Sin + hidden matmuls (range-based dep tracking)
                    nc.sync.dma_start(cbuf[:, 0:NC], src[:, 0:NC])
                    nc.sync.dma_start(cbuf[:, 4 * NC:4 * NC + 512],
                                      src[:, 4 * NC:4 * NC + 512])
                    nc.sync.dma_start(cbuf[:, NC:4 * NC], src[:, NC:4 * NC])
                    nc.sync.dma_start(cbuf[:, 4 * NC + 512:],
                                      src[:, 4 * NC + 512:])
                else:
                    nc.sync.dma_start(cbuf[:], src)
                r_m = io.tile([128, NC], f32, tag="rm")
                nc.sync.dma_start(r_m[:], inp[C * 128:(C + 1) * 128, :])
                # host-prebaked s-dup'd a|b|c' rows, straight from HBM on the
                # scalar HWDGE queue -- issued while ACT idles waiting for the
                # first relu-drain, so neither the sync queue nor the ACT
                # compute stream pays for it.
                abc_C = abcp.tile([128, 3 * NC], f32, tag="abcC")
                nc.scalar.dma_start(abc_C[:], abc[C * 128:(C + 1) * 128, :])
                t["r_m"], t["cbuf"] = r_m, cbuf
                t["abc"] = abc_C
                return t

            def block_sin_closures(t):
                cbuf = t["cbuf"]

                def mk(g):
                    def sg():
                        nc.scalar.activation(cbuf[:, g * NC:(g + 1) * NC],
                                             cbuf[:, g * NC:(g + 1) * NC],
                                             AF.Sin, bias=cb_t[:, 5:6],
                                             scale=PI4)
                    return sg

                return [mk(g) for g in range(4)]

            # keep the sync queue lean at startup (only cb + chunk-0 splits):
            # wsig/wsum ride the scalar queue ahead of the bulky abc block
            wsig_t = cpool.tile([128, 512], bf16)
            nc.scalar.dma_start(wsig_t[:], bslice(O_WSIG, 128, 512))
            wsum_t = cpool.tile([128, CHUNKS * 128], bf16)
            nc.scalar.dma_start(wsum_t[:], bslice(O_WSUM, 128, CHUNKS * 128))

            cur = issue_loads(0)
            for f in block_sin_closures(cur):
                f()

            # Deferred-emission schedule: each half's mask chain, mask trig,
            # next chunk's block Sins, and the weighted-sum matmul are fed
            # ONE PER zmm ITERATION into the NEXT half's stream, so the ACT /
            # DVE queues never burst at a half boundary and the first drains
            # of each half are always at the head of their engine's FIFO.
            def make_deferred(fs, ct_m, st_m, abc_C, hold, C):
                st = {}

                def d_q1():
                    q1 = work.tile([128, HF], f32, tag="q1")
                    nc.vector.scalar_tensor_tensor(
                        q1[:], ct_m[:, fs:fs + HF], 1.0,
                        abc_C[:, fs:fs + HF], ALU.mult, ALU.mult)
                    st['q1'] = q1

                def d_q2():
                    q2 = work.tile([128, HF], f32, tag="q2")
                    nc.vector.scalar_tensor_tensor(
                        q2[:], st_m[:, fs:fs + HF], 1.0,
                        abc_C[:, NC + fs:NC + fs + HF], ALU.mult, ALU.mult)
                    st['q2'] = q2

                def d_q3():
                    q3 = work.tile([128, HF], f32, tag="q3")
                    nc.vector.tensor_add(q3[:], st['q1'][:], st['q2'][:])
                    st['q3'] = q3

                def d_msk():
                    msk = work.tile([128, HF], bf16, tag="msk")
                    nc.vector.tensor_tensor(
                        msk[:], st['q3'][:],
                        abc_C[:, 2 * NC + fs:2 * NC + fs + HF], ALU.is_gt)
                    st['msk'] = msk

                def d_tm():
                    # vis*msk = 0.5*(tanh+1)*msk, fused
                    tm = work.tile([128, HF], bf16, tag="tm")
                    nc.vector.scalar_tensor_tensor(
                        tm[:], hold['t'][:], 1.0, st['msk'][:],
                        ALU.add, ALU.mult)
                    st['tm'] = tm

                def d_sum():
                    nc.tensor.matmul(
                        out_ps[:, fs:fs + HF],
                        wsum_t[:, C * 128:(C + 1) * 128], st['tm'][:],
                        start=(C == 0), stop=(C == CHUNKS - 1))
                    if C == CHUNKS - 1:
                        out_sb = cpool.tile([128, HF], f32, tag=f"osb{fs}")
                        nc.vector.tensor_copy(out_sb[:], out_ps[:, fs:fs + HF])
                        nc.sync.dma_start(out[:, fs:fs + HF], out_sb[:])

                return [d_q1, d_q2, d_q3, d_msk, d_tm, d_sum]

            pending = []

            for C in range(CHUNKS):
                nxt = issue_loads(C + 1) if C + 1 < CHUNKS else None
                cbuf = cur["cbuf"]
                abc_C = cur["abc"]
                r_m = cur["r_m"]
                last = C == CHUNKS - 1

                ct_m = trig.tile([128, NC], f32, tag="ct")
                st_m = trig.tile([128, NC], f32, tag="st")

                def sin_ct(ct_m=ct_m, r_m=r_m):
                    nc.scalar.activation(ct_m[:], r_m[:], AF.Sin,
                                         bias=cb_t[:, 0:1], scale=PI4)

                def sin_st(st_m=st_m, r_m=r_m):
                    nc.scalar.activation(st_m[:], r_m[:], AF.Sin,
                                         bias=cb_t[:, 1:2], scale=PI4)

                for hf in range(2):
                    fs = hf * HF
                    if hf == 0:
                        extras = [sin_ct, sin_st] + pending
                    else:
                        bs = block_sin_closures(nxt) if nxt is not None else []
                        pd = list(pending)
                        extras = []
                        for i in range(max(len(bs), len(pd))):
                            if i < len(bs):
                                extras.append(bs[i])
                            if i < len(pd):
                                extras.append(pd[i])
                    pending = []
                    zt = zps.tile([128, HF], f32, tag="zt")
                    hrs = [None] * LPC
                    # at hf1 the ACT queue starts with the block Sins; give
                    # the first drains to DVE there so zmm never waits
                    drain_eng = DRAIN if hf == 0 else DRAIN[::-1]
                    def hidden(j16):
                        # 32-row K-band at array band 32*j': four lobes of a
                        # group run concurrently in the PE array
                        g, jp = divmod(j16, 4)
                        b = 32 * jp
                        ph = ps.tile([128, HF], f32, tag="ph")
                        wc = 4 * NC + g * 512 + jp * 128
                        nc.tensor.matmul(ph[:],
                                         cbuf[b:b + 32, wc:wc + 128],
                                         cbuf[b:b + 32, g * NC + fs:
                                              g * NC + fs + HF],
                                         start=True, stop=True,
                                         tile_position=(b, 0))
                        hr = hrp.tile([128, HF], bf16, tag="hr")
                        if drain_eng[j16] == "A":
                            nc.scalar.activation(hr[:], ph[:], AF.Relu,
                                                 bias=cb_t[:, 3:4])
                        else:
                            nc.vector.tensor_scalar(hr[:], ph[:], 0.0, 0.0,
                                                    ALU.max, ALU.bypass)
                        hrs[j16] = hr

                    def zmm(j16):
                        j = j16 % 8
                        g = j16 // 8
                        nc.tensor.matmul(zt[64 * g:64 * (g + 1), :],
                                         wsig_t[:, j * 64:(j + 1) * 64],
                                         hrs[j16][:], start=(j == 0),
                                         stop=(j == 7))

                    final = last and hf == 1
                    mask_defer = None
                    hold = {}
                    for j16 in range(4):
                        hidden(j16)
                    for j16 in range(LPC):
                        if j16 % 4 == 0 and j16 + 4 < LPC:
                            # issue the next 4-lobe group as one block so its
                            # row-tiled matmuls overlap in the array
                            for jj in range(4):
                                hidden(j16 + 4 + jj)
                        zmm(j16)
                        if extras and j16 >= 1:
                            extras.pop(0)()
                        if final and j16 >= LPC - 4:
                            # final half: its own mask chain runs inline so
                            # only tanh+tm+sum trail the last zmm
                            if mask_defer is None:
                                mask_defer = make_deferred(fs, ct_m, st_m,
                                                           abc_C, hold, C)
                            mask_defer.pop(0)()
                    tanhv = work.tile([128, HF], bf16, tag="tanhv")
                    nc.scalar.activation(tanhv[:], zt[:], AF.Tanh,
                                         bias=cb_t[:, 2:3], scale=0.5)
                    hold['t'] = tanhv
                    if final:
                        for f in mask_defer:
                            f()
                    else:
                        pending = make_deferred(fs, ct_m, st_m, abc_C,
                                                hold, C)

                if nxt is not None:
                    cur = nxt

    nc.compile()
    return nc


def _host_constants(points, normals, root_rot, lgtSGLobes, lgtSGLambdas,
                    W1, b1, W2, b2):
    f8 = np.float64
    lob = lgtSGLobes.astype(f8)
    l = lob / (np.linalg.norm(lob, axis=-1, keepdims=True) + TINY)
    z = np.zeros_like(l)
    z[:, 2] = 1.0
    U = np.cross(z, l)
    U = U / (np.linalg.norm(U, axis=-1, keepdims=True) + TINY)
    V = np.cross(l, U)
    V = V / (np.linalg.norm(V, axis=-1, keepdims=True) + TINY)
    sharp = lgtSGLambdas[:, 0].astype(f8)
    r_phi = np.minimum(np.arccos(1.0 - 1.0 / sharp), np.pi / 3.0)
    sp, cp = np.sin(r_phi), np.cos(r_phi)

    Wd = root_rot.astype(f8) @ W1[3:].astype(f8)          # [3,H]
    A = sp[:, None] * (U @ Wd)                             # [L,H]
    B = sp[:, None] * (V @ Wd)
    Cc = cp[:, None] * (l @ Wd)
    W1p = W1[:3].astype(f8)                                # [3,H]
    b1f = b1.astype(f8)
    w2 = W2[:, 0].astype(f8)
    w_l = np.exp(sharp * (cp - 1.0))
    scale_l = w_l / (S * w_l + TINY)
    spU = sp[:, None] * U
    spV = sp[:, None] * V
    cpl = cp[:, None] * l

    # wcst: per chunk [128, 2048]; lobe pos (g,jp) lives in partition band
    # 32*jp, cols g*512 + jp*128 (the [32,128] row-tile stationary).
    # Band rows: 0..7 ct -> -A, 8..15 st -> -B, 16..18 -> W1p*PS,
    # 19 -> (b1-C)*PS (the asin-coded rows decode to p/PS and 1/PS).
    PS = float(max(np.abs(points).max() * 1.0001, 1.0))
    wcstZ = np.zeros((CHUNKS, 128, 2048), f8)
    wcstV = wcstZ.reshape(CHUNKS, 128, 4, 4, 8, H)  # [C, row, g, jp, s, h]
    for ll in range(L):
        C, pos = divmod(ll, LPC)
        g, jp = divmod(pos, 4)
        b = 32 * jp
        for s in range(8):
            wcstV[C, b + s, g, jp, s, :] = -A[ll]
            wcstV[C, b + 8 + s, g, jp, s, :] = -B[ll]
        for d in range(3):
            wcstV[C, b + 16 + d, g, jp, :, :] = W1p[d] * PS
        wcstV[C, b + 19, g, jp, :, :] = (b1f - Cc[ll])[None, :] * PS

    # mask dots in [L, N] layout, f64 math rounded once to f32; c' = TINY - c
    # so the device compare stays a single is_gt.  (s-dup happens per core in
    # _make_in_maps.)
    nT = normals.astype(f8).T                              # [3, N]
    abc_a = (spU @ nT).astype(np.float32)                  # [L, N]
    abc_b = (spV @ nT).astype(np.float32)
    abc_c = (TINY - (cpl @ nT)).astype(np.float32)

    # wsig: [128, 8*64]; for in-group position p: cols p*64 + l''*8 + s' =
    # w2[h]*delta(s,s')*delta(l'',p)
    wsig = np.zeros((8, H, 8, 8, 8), f8)
    for p in range(8):
        for s in range(8):
            wsig[s, :, p, p, s] = w2
    # wsum: per-chunk [128, L] blocks with HALF the scale (tanh folding);
    # block cc maps chunk-local lobe lp to global output column cc*16+lp.
    wsum = np.zeros((LPC, 8, CHUNKS, L), f8)
    for cc in range(CHUNKS):
        for lp in range(LPC):
            wsum[lp, :, cc, cc * LPC + lp] = 0.5 * scale_l[cc * LPC + lp]

    cbias = np.zeros((128, 8), f8)
    s_of_p = np.arange(128) % 8
    # ACT Sin LUT domain is [-pi, pi]; input is r*pi/4 + bias with r in [0,1),
    # so shift each s-row by a full period where needed to stay in range.
    cos_bias = s_of_p * (np.pi / 4.0) + np.pi / 2.0 - 2.0 * np.pi * (s_of_p >= 2)
    sin_bias = s_of_p * (np.pi / 4.0) - 2.0 * np.pi * (s_of_p >= 4)
    cbias[:, 0] = cos_bias
    cbias[:, 1] = sin_bias
    cbias[:, 2] = float(b2[0]) * 0.5                      # tanh bias = b2/2
    cbias[:, 3] = 0.0                                     # relu bias
    # col5: group-tile Sin bias, mod-32 band pattern [ct8|st8|pc+pad 0]
    r32 = np.arange(128) % 32
    cbias[:, 5] = np.where(r32 < 8, cos_bias[r32 % 8],
                           np.where(r32 < 16, sin_bias[r32 % 8], 0.0))

    return dict(wcst=wcstZ.reshape(CHUNKS, 128, 2048), PS=PS,
                abc_a=abc_a, abc_b=abc_b, abc_c=abc_c,
                wsig=wsig.reshape(128, 512),
                wsum=wsum.reshape(128, CHUNKS * L), cb=cbias)


def _make_in_maps(inputs):
    import ml_dtypes
    bf16 = np.dtype(ml_dtypes.bfloat16)
    f32 = np.float32

    const = _host_constants(inputs["points"], inputs["normals"],
                            inputs["root_rot"], inputs["lgtSGLobes"],
                            inputs["lgtSGLambdas"], inputs["W1"],
                            inputs["b1"], inputs["W2"], inputs["b2"])

    # replicated bf16 constant tail of `big`
    wtail = np.concatenate([const["wsig"].ravel(),
                            const["wsum"].ravel()]).astype(bf16)
    wcst = const["wcst"].astype(bf16)                      # [CHUNKS, 128, 2048]

    r_t = np.asarray(inputs["r_theta_random"], f32).transpose(1, 2, 0).reshape(L * S, N)
    pT = np.asarray(inputs["points"], np.float64).T
    PS = const["PS"]

    # asin-coded pc rows: Sin(pc*pi/4) on device recovers p/PS resp. 1/PS
    pc4 = np.empty((4, N), np.float64)
    pc4[0:3] = np.arcsin(pT / PS) * (4.0 / np.pi)
    pc4[3] = np.arcsin(1.0 / PS) * (4.0 / np.pi)

    # group tiles: per chunk 4 tiles [128, N]; lobe j' of a group owns rows
    # 32*j'+[0..31] = [theta(cos)8 | theta(sin)8 | pc4 | zeros12]
    inpb = np.zeros((CHUNKS * 4 * 128, N), f32)
    gv = inpb.reshape(CHUNKS, 4, 4, 32, N)                 # [C, g, j', row, n]
    for ll in range(L):
        C, pos = divmod(ll, LPC)
        g, jp = divmod(pos, 4)
        slab = r_t[ll * 8:(ll + 1) * 8]                    # [8, N]
        gv[C, g, jp, 0:8] = slab
        gv[C, g, jp, 8:16] = slab
        gv[C, g, jp, 16:20] = pc4
    inpb = inpb.astype(bf16)                               # [CHUNKS*512, N]

    in_maps = []
    for c in range(NCORES):
        sl = slice(c * NC, (c + 1) * NC)
        inp = np.zeros((INP_ROWS, NC), f32)
        inp[R_RT:R_RT + L * S] = r_t[:, sl]
        inp[R_CB:R_CB + 128, 0:8] = const["cb"]
        abc = np.empty((CHUNKS * 128, 3 * NC), f32)
        for C in range(CHUNKS):
            rows = slice(C * 128, (C + 1) * 128)
            ls = slice(C * LPC, (C + 1) * LPC)
            abc[rows, 0:NC] = np.repeat(const["abc_a"][ls, sl], 8, axis=0)
            abc[rows, NC:2 * NC] = np.repeat(const["abc_b"][ls, sl], 8, axis=0)
            abc[rows, 2 * NC:] = np.repeat(const["abc_c"][ls, sl], 8, axis=0)
        big = np.empty(BIG_ELEMS, bf16)
        bufv = big[O_BLK:O_WSIG].reshape(CHUNKS, 128, CB_COLS)
        core_blk = np.asarray(inpb[:, sl]).reshape(CHUNKS, 4, 128, NC)
        for C in range(CHUNKS):
            for g in range(4):
                bufv[C, :, g * NC:(g + 1) * NC] = core_blk[C, g]
            bufv[C, :, 4 * NC:] = wcst[C]
        big[O_WSIG:] = wtail
        in_maps.append({
            "inp": np.ascontiguousarray(inp),
            "big": big,
            "abc": abc,
        })
    return in_maps


def kernel(points, normals, root_rot, lgtSGLobes, lgtSGLambdas,
           r_theta_random, W1, b1, W2, b2):
    global _PROG
    from concourse.bass_utils import run_bass_kernel_spmd

    if _PROG is None:
        _PROG = _build_program()
    nc = _PROG

    in_maps = _make_in_maps(dict(
        points=points, normals=normals, root_rot=root_rot,
        lgtSGLobes=lgtSGLobes, lgtSGLambdas=lgtSGLambdas,
        r_theta_random=r_theta_random, W1=W1, b1=b1, W2=W2, b2=b2))

    res = run_bass_kernel_spmd(nc, in_maps, list(range(NCORES)))

    f32 = np.float32
    out_full = np.empty((N, L), f32)
    for c in range(NCORES):
        out_full[c * NC:(c + 1) * NC, :] = res.results[c]["out"].T
    return out_full

